# revision 5
# baseline (speedup 1.0000x reference)
"""CapsuleLayer (dynamic routing, 3 iterations) on 8 Trainium2 NeuronCores.

Strategy (N-sharded, j-major, bf16):
  - Each core owns 144 of the 1152 input capsules.  W DMA per core is 1/8th
    of the full tensor in bf16; cross-core traffic is 3 AllReduces of the
    routing sums (fp16 payloads, batch-quad split so they overlap compute).
  - u_hat built by PE matmuls: stationary = block-diagonal inputs pack
    (K = 8 n's x 16 i = 128), moving = W pack [128, (j,c) j-major].  Output
    partitions = (n8, b16-quad).  PSUM evacuated to SBUF bf16, split between
    the Vector and Activation engines.  The s0 matmuls run first so the
    first AllReduce launches ~1/3 into the build and overlaps the rest.
  - u_hat SBUF layout is half-major: each batch-quad-pair sweep sees
    contiguous 2-group slabs, so every DVE op covers 4096 elements with a
    packed last dim (2x mode): t1 = u*v, a 5-level tree reduce over j,
    t2 = u*c with the j-broadcast in a middle AP dim.
  - Softmax runs on the Activation engine (exp with accum_out; coupling =
    copy-with-per-partition-scale).  s = delta^T @ t2 PE matmuls accumulate
    over groups in PSUM.
  - The final reduce + squash happen on host (partials are summed there),
    saving a third of the collective traffic.
"""

import os
import numpy as np
from contextlib import ExitStack

import ml_dtypes

import concourse.bass as bass
import concourse.mybir as mybir
from concourse import tile
from concourse.bass_utils import run_bass_kernel_spmd
from concourse.vector_clock import ScopedClock

# Problem constants
B, N, Di = 64, 1152, 16
C, Dc = 32, 32
NCORES = 8
NLOC = N // NCORES          # 144 input capsules per core
NG = NLOC // 8              # 18 groups of 8 n's
NQ = 4                      # four b-quads of 16
EPS = 1e-7

F32 = mybir.dt.float32
F16 = mybir.dt.float16

# 16-bit dtype switch (bf16 default: measured faster than fp16 on HW DVE)
_DT = {"dev": mybir.dt.bfloat16, "np": ml_dtypes.bfloat16}


def set_dtype16(name):
    if name == "bf16":
        _DT["dev"] = mybir.dt.bfloat16
        _DT["np"] = ml_dtypes.bfloat16
    else:
        _DT["dev"] = mybir.dt.float16
        _DT["np"] = np.float16


GP_T2_MOD = 4               # t2 goes to gpsimd for (g % GP_T2_MOD) != 0


class PatchedTC(tile.TileContext):
    """This walrus build only supports ONE sync-wait per instruction; Tile's
    final drain carries one wait per outstanding DMA-queue semaphore.  Split
    the extras onto single-wait SP nops."""

    def _drain_and_barrier(self, tick_clock, wait_clock):
        nc = self.nc
        drain_inst = nc.sync.drain()
        wait_clock.add_sem_waits(
            drain_inst.ins, ScopedClock({None: tick_clock.global_clock})
        )
        si = drain_inst.ins.sync_info
        if si is not None and len(si.on_wait) > 1:
            waits = list(si.on_wait)
            del si.on_wait[1:]
            for w in waits[1:]:
                n2 = nc.sync.nop()
                if n2.ins.sync_info is None:
                    n2.ins.sync_info = mybir.SyncInfo(on_update=[], on_wait=[w])
                else:
                    n2.ins.sync_info.on_wait.append(w)
        nc.all_engine_barrier()
        popped = nc._tile_sem_poison_stack.pop()
        assert popped is self._sem_poison
        nc.clear_and_free_semaphores(list(self.sems.allocated().values()))
        nc.all_engine_barrier()


def _split_multi_waits(nc):
    """Post-pass: any instruction carrying >1 sync wait gets the extras moved
    onto same-engine nop instructions inserted right before it."""
    for fn in nc.m.functions:
        for bb in fn.blocks:
            insts = list(bb.instructions)
            out = []
            for ins in insts:
                si = getattr(ins, "sync_info", None)
                if si is not None and si.on_wait is not None and len(si.on_wait) > 1:
                    waits = list(si.on_wait)
                    del si.on_wait[1:]
                    for k, w in enumerate(waits[1:]):
                        nop = mybir.InstNoOp(
                            name=f"{ins.name}-wsplit{k}", ins=[], outs=[]
                        )
                        nop.engine = ins.engine
                        nop.sync_info = mybir.SyncInfo(on_update=[], on_wait=[w])
                        out.append(nop)
                out.append(ins)
            if len(out) != len(insts):
                bb.instructions[:] = out


def _bcast_j(ap, j=32):
    """[P, C] AP -> [P, C, j(step 0)] broadcast view (c-major layouts)."""
    lst = [list(p) for p in ap.ap]
    new = [lst[0], lst[-1], [0, j]]
    return bass.AP(ap.tensor, ap.offset, new)


def _view_cj(ap, c=32, j=32):
    """[P, c*j] AP (c-major, j contiguous) -> [P, c, j]."""
    return ap.rearrange("p (c j) -> p c j", c=c, j=j)


def build_program(repeat=1, no_ar=False, hwloop=0, gp_mod=GP_T2_MOD,
                  safe_act=False, ar_f32=False, no_gp=True, gp_evac=False,
                  phases=99, ar2_full=False):
    nc = bass.Bass()
    F16 = _DT["dev"]        # bulk dtype (u, t1/t2, tree, logits, coupling)
    F16P = mybir.dt.float16  # precision dtype (AR payload, squash chain)

    w_pack = nc.declare_dram_parameter("w_pack", [NG, 128, 1024], F16, isOutput=False)
    x_bd = nc.declare_dram_parameter("x_bd", [NG, 128, NQ * 128], F16, isOutput=False)
    x_dense = nc.declare_dram_parameter("x_dense", [128, NG * 64], F16, isOutput=False)
    delta = nc.declare_dram_parameter("delta", [128, NQ * 64], F16, isOutput=False)
    out_ext = nc.declare_dram_parameter("out", [B, 1024], F16P, isOutput=True)

    ctx = ExitStack()
    lowp = nc.allow_low_precision(reason="fp16 kernel: values are O(1)")
    lowp.__enter__()
    with PatchedTC(nc) as tc, ctx:
        sb = ctx.enter_context(tc.tile_pool(name="sb", bufs=1))
        wpool = ctx.enter_context(tc.tile_pool(name="w", bufs=2))
        xpool = ctx.enter_context(tc.tile_pool(name="x", bufs=3))
        psum_u = ctx.enter_context(tc.tile_pool(name="psu", bufs=3, space="PSUM"))
        psum_s = ctx.enter_context(tc.tile_pool(name="pss", bufs=1, space="PSUM"))
        tpool = ctx.enter_context(tc.tile_pool(name="t", bufs=1))
        trpool = ctx.enter_context(tc.tile_pool(name="tr", bufs=1))
        smpool = ctx.enter_context(tc.tile_pool(name="sm", bufs=4))
        dram = ctx.enter_context(tc.tile_pool(name="dram", bufs=1, space="DRAM"))

        # Persistent SBUF
        u_sb = sb.tile([128, NG * NQ * 1024], F16, tag="uhat")      # 144 KB/part
        b1_sb = sb.tile([128, NG * NQ * 32], F16, tag="b1")         # 4.5 KB/part
        delta_sb = sb.tile([128, NQ * 64], F16, tag="delta")
        vb_sb = sb.tile([128, NQ * 1024], F16P, tag="vbcast")        # 8 KB/part
        s16_sb = sb.tile([64, 1024], F16P, tag="s16")
        ar16_sb = sb.tile([64, 1024], F16P, tag="ar16")
        if ar_f32:
            ar32_sb = sb.tile([64, 1024], F32, tag="ar32")
            s32_sb = sb.tile([64, 1024], F32, tag="s32")
        else:
            ar32_sb = s32_sb = None
        v16_sb = sb.tile([64, 1024], F16P, tag="v16")
        s_sb = sb.tile([64, 1024], F16P, tag="sfull")
        sq16_sb = sb.tile([64, 1024], F16P, tag="sq")
        sc16_sb = sb.tile([64, 32], F16P, tag="sc16")
        n2_sb = sb.tile([64, 96], F32, tag="n2")  # [0:32] n2 | [32:64] scratch

        nc.sync.dma_start(out=delta_sb[:], in_=delta[:])
        xd_sb = sb.tile([128, NG * 64], F16, tag="xd")          # 2.25 KB/part
        nc.sync.dma_start(out=xd_sb[:], in_=x_dense[:])

        def u_off(g, q):
            # half-major: [half][g][q%2] so each sweep half sees contiguous
            # 2-group slabs
            return (q // 2) * (NG * 2048) + g * 2048 + (q % 2) * 1024

        def u_slice(g, q):
            off = u_off(g, q)
            return u_sb[:, off:off + 1024]

        def b1_off(g, q):
            return (q // 2) * (NG * 64) + g * 64 + (q % 2) * 32

        def u_matmul_evac(g, q, w_t, xb4_t):
            ps_u = psum_u.tile([128, 1024], F32, tag="u")
            for h in range(2):
                nc.tensor.matmul(
                    ps_u[:, h * 512:(h + 1) * 512],
                    xb4_t[:, q * 128:(q + 1) * 128],
                    w_t[:, h * 512:(h + 1) * 512],
                    start=True, stop=True,
                )
            usl = u_slice(g, q)
            if gp_evac:
                nc.vector.tensor_copy(usl[:, 0:352], ps_u[:, 0:352])
                nc.scalar.copy(usl[:, 352:688], ps_u[:, 352:688])
                nc.gpsimd.tensor_copy(usl[:, 688:1024], ps_u[:, 688:1024])
            else:
                # DVE is a bit slower per el from PSUM: give it the smaller cut
                nc.vector.tensor_copy(usl[:, 0:448], ps_u[:, 0:448])
                nc.scalar.copy(usl[:, 448:1024], ps_u[:, 448:1024])

        # ---------- Build: pass A (s0 + quad 0), pass B (quads 1-3) ----------
        def build_pass_a():
            ps_s0 = psum_s.tile([64, 1024], F32, tag="s")
            for g in range(NG):
                w_t = wpool.tile([128, 1024], F16, tag="w")
                nc.sync.dma_start(out=w_t[:], in_=w_pack[g])
                xb4_t = xpool.tile([128, 512], F16, tag="xb")
                nc.sync.dma_start(out=xb4_t[:], in_=x_bd[g])
                for h in range(2):
                    nc.tensor.matmul(
                        ps_s0[:, h * 512:(h + 1) * 512],
                        xd_sb[:, g * 64:(g + 1) * 64],
                        w_t[:, h * 512:(h + 1) * 512],
                        start=(g == 0), stop=(g == NG - 1),
                    )
                u_matmul_evac(g, 0, w_t, xb4_t)
            return ps_s0

        def build_pass_b():
            for g in range(NG):
                w_t = wpool.tile([128, 1024], F16, tag="w")
                nc.sync.dma_start(out=w_t[:], in_=w_pack[g])
                xb4_t = xpool.tile([128, 512], F16, tag="xb")
                nc.sync.dma_start(out=xb4_t[:], in_=x_bd[g])
                for q in range(1, NQ):
                    u_matmul_evac(g, q, w_t, xb4_t)

        # ---------- AllReduce + squash ----------
        def allreduce_pre(rows=slice(0, 64)):
            """Ship already-filled ar_src rows through the AllReduce."""
            nr = rows.stop - rows.start
            ar_dt = F32 if ar_f32 else F16P
            ar_src = ar32_sb if ar_f32 else ar16_sb
            bounce_in = dram.tile([64, 1024], ar_dt, tag=f"cin{rows.start}")
            bounce_out = dram.tile([64, 1024], ar_dt, tag=f"cout{rows.start}")
            nc.sync.dma_start(out=bounce_in[0:nr], in_=ar_src[rows])
            if no_ar:
                nc.sync.dma_start(out=bounce_out[0:nr], in_=bounce_in[0:nr])
            else:
                nc.gpsimd.collective_compute(
                    "AllReduce",
                    mybir.AluOpType.add,
                    replica_groups=[list(range(NCORES))],
                    ins=[bounce_in[0:nr]],
                    outs=[bounce_out[0:nr]],
                )
            nc.sync.dma_start(
                out=(s32_sb if ar_f32 else s16_sb)[rows], in_=bounce_out[0:nr]
            )

        def allreduce(ps_ap, scale0, rows=slice(0, 64)):
            """ps_ap [nr,1024] PSUM rows -> scaled -> AllReduce -> s16_sb rows."""
            ar_src = ar32_sb if ar_f32 else ar16_sb
            nc.vector.tensor_scalar(
                ar_src[rows], ps_ap, scale0, None, mybir.AluOpType.mult
            )
            allreduce_pre(rows)

        def squash(rows=slice(0, 64)):
            """post-AR s rows -> v16_sb rows (fp16).  Scalar chain folded onto
            the Activation engine where possible; fp16 squares for DVE 2x."""
            s16 = (s32_sb if ar_f32 else s16_sb)[rows]
            sq = sq16_sb[rows]
            nc.vector.tensor_mul(sq, s16, s16)
            sqv = sq.rearrange("p (j c) -> p j c", c=32)
            nc.vector.tensor_add(sqv[:, 0:16], sqv[:, 0:16], sqv[:, 16:32])
            nc.vector.tensor_add(sqv[:, 0:8], sqv[:, 0:8], sqv[:, 8:16])
            nc.vector.tensor_add(sqv[:, 0:4], sqv[:, 0:4], sqv[:, 4:8])
            nc.vector.tensor_add(sqv[:, 0:2], sqv[:, 0:2], sqv[:, 2:4])
            n2 = n2_sb[rows, 0:32]
            nc.vector.tensor_add(n2, sq[:, 0:32], sq[:, 32:64])
            rt = n2_sb[rows, 32:64]
            p1 = n2_sb[rows, 64:96]
            nc.vector.tensor_scalar(rt, n2, EPS, None, mybir.AluOpType.add)
            nc.scalar.activation(rt, rt, mybir.ActivationFunctionType.Sqrt)
            nc.vector.tensor_scalar(p1, n2, 1.0, None, mybir.AluOpType.add)
            nc.vector.tensor_mul(rt, rt, p1)
            nc.vector.reciprocal(rt, rt)
            sc16 = sc16_sb[rows]
            nc.vector.tensor_mul(sc16, n2, rt)   # fp16 scale factor
            sclst = [list(d) for d in sc16.ap]
            scbc = bass.AP(sc16.tensor, sc16.offset,
                           [sclst[0], [0, 32], [1, 32]])
            nc.vector.tensor_tensor(
                v16_sb[rows].rearrange("p (j c) -> p j c", c=32),
                s16.rearrange("p (j c) -> p j c", c=32),
                scbc, mybir.AluOpType.mult,
            )

        vdram = dram.tile([64, 1024], F16P, tag="vd")

        def bcast_v(quads):
            """v16_sb rows -> vb_sb quad blocks (replicate over n8) via a DRAM
            bounce: 1 + len(quads) DMAs instead of 8*len(quads)."""
            r0, r1 = quads[0] * 16, (quads[-1] + 1) * 16
            nc.sync.dma_start(out=vdram[r0:r1], in_=v16_sb[r0:r1])
            for q in quads:
                vq = vdram[q * 16:(q + 1) * 16]
                rep = bass.AP(vq.tensor, vq.offset, [[0, 8], [1024, 16], [1, 1024]])
                nc.sync.dma_start(
                    out=vb_sb[:, q * 1024:(q + 1) * 1024], in_=rep
                )

        # ---------- Sweep (per quad-pair half, 2-group slabs) ----------
        NPAIR = NG // 2

        def sweep_half(is_b, quads, tag):
            """Process quads `quads` (a contiguous pair) for all groups, two
            groups per slab.  Returns PSUM [nr,1024] with the s partial for
            those batch rows."""
            r0 = quads[0] * 16
            nr = len(quads) * 16
            nq = len(quads)
            q0 = quads[0]
            half = q0 // 2
            hbase = half * (NG * 2048)
            ps_s = psum_s.tile([64, 1024], F32, tag="s")
            first = [True, True]

            vhalf = vb_sb[:, q0 * 1024:(q0 + nq) * 1024]
            vlst = [list(d) for d in vhalf.ap]
            # [p, 2048] -> [p, grep(0-step 2), 2048]: periodic over the 2
            # groups of a slab
            vrep = bass.AP(vhalf.tensor, vhalf.offset,
                           [vlst[0], [0, 2], [1, 2048]])

            for gp_i in range(NPAIR):
                g0 = 2 * gp_i
                if is_b:
                    b2_g = smpool.tile([128, 128], F16, tag="b2")
                uslab = u_sb[:, hbase + g0 * 2048:hbase + g0 * 2048 + 4096]
                u3 = uslab.rearrange("p (g f) -> p g f", g=2)
                t1 = tpool.tile([128, 4096], F16, tag="t1")
                nc.vector.tensor_tensor(
                    t1[:].rearrange("p (g f) -> p g f", g=2), u3, vrep,
                    mybir.AluOpType.mult,
                )
                # tree over j (outer dim; c stays contiguous in the last
                # dim so every op takes the DVE 2x path)
                t1v = t1[:].rearrange("p (m j c) -> p m j c", m=4, c=32)
                l1 = trpool.tile([128, 2048], F16, tag="l1")
                l1v = l1[:].rearrange("p (m j c) -> p m j c", m=4, c=32)
                nc.vector.tensor_add(l1v, t1v[:, :, 0:16], t1v[:, :, 16:32])
                l2 = trpool.tile([128, 1024], F16, tag="l2")
                l2v = l2[:].rearrange("p (m j c) -> p m j c", m=4, c=32)
                nc.vector.tensor_add(l2v, l1v[:, :, 0:8], l1v[:, :, 8:16])
                l3 = trpool.tile([128, 512], F16, tag="l3")
                l3v = l3[:].rearrange("p (m j c) -> p m j c", m=4, c=32)
                nc.vector.tensor_add(l3v, l2v[:, :, 0:4], l2v[:, :, 4:8])
                l4 = trpool.tile([128, 256], F16, tag="l4")
                l4v = l4[:].rearrange("p (m j c) -> p m j c", m=4, c=32)
                nc.vector.tensor_add(l4v, l3v[:, :, 0:2], l3v[:, :, 2:4])
                bsl = b1_sb[:, half * NG * 64 + g0 * 64:half * NG * 64 + g0 * 64 + 128]
                dst = b2_g[:] if is_b else bsl
                dstv = dst.rearrange("p (m c) -> p m c", c=32)
                nc.vector.tensor_add(dstv, l4v[:, :, 0], l4v[:, :, 1])
                if is_b:
                    nc.vector.tensor_add(b2_g[:], b2_g[:], bsl)
                    logits_g = b2_g[:]
                else:
                    logits_g = bsl
                # softmax over c (Activation engine), per (g,q) 32-col block
                e_g = smpool.tile([128, 128], F32, tag="e")
                z_g = smpool.tile([128, 4], F32, tag="z")
                for k in range(4):
                    nc.scalar.activation(
                        e_g[:, k * 32:(k + 1) * 32],
                        logits_g[:, k * 32:(k + 1) * 32],
                        mybir.ActivationFunctionType.Exp,
                        accum_out=z_g[:, k:k + 1],
                    )
                r_g = smpool.tile([128, 4], F32, tag="r")
                nc.vector.reciprocal(r_g[:], z_g[:])
                c_g = smpool.tile([128, 128], F16, tag="c")
                for k in range(4):
                    nc.scalar.mul(
                        c_g[:, k * 32:(k + 1) * 32],
                        e_g[:, k * 32:(k + 1) * 32],
                        r_g[:, k:k + 1],
                    )
                # t2 slab: u * c broadcast over inner j
                t2 = tpool.tile([128, 4096], F16, tag="t2")
                # c_g [p,(m4,c32)] -> [p, m4, j32(step 0), c32]: broadcast in a
                # middle dim keeps the last dim packed (2x path)
                cap = c_g[:].ap
                cbc = bass.AP(c_g[:].tensor, c_g[:].offset,
                              [list(cap[0]), [32, 4], [0, 32], [1, 32]])
                nc.vector.tensor_tensor(
                    t2[:].rearrange("p (m j c) -> p m j c", m=4, c=32),
                    uslab.rearrange("p (m j c) -> p m j c", m=4, c=32),
                    cbc, mybir.AluOpType.mult,
                )
                # s += delta_q^T @ t2 per (g, quad)
                for gg in range(2):
                    for qi, q in enumerate(quads):
                        dl = delta_sb[:, q * 64 + r0:q * 64 + r0 + nr]
                        last = (gp_i == NPAIR - 1) and (gg == 1) and (qi == nq - 1)
                        off = gg * 2048 + qi * 1024
                        for h in range(2):
                            nc.tensor.matmul(
                                ps_s[0:nr, h * 512:(h + 1) * 512], dl,
                                t2[:, off + h * 512:off + (h + 1) * 512],
                                start=first[h], stop=last,
                            )
                            first[h] = False
            return ps_s

        # ---------- One full routing pass ----------
        def one_pass():
            ps_s0 = build_pass_a()
            allreduce(ps_s0[:], 1.0 / C)
            build_pass_b()
            squash()
            bcast_v(range(NQ))
            if phases <= 0:
                nc.sync.dma_start(out=out_ext[:], in_=s16_sb[:])
                return
            # Sweep A in halves; AR2 per half overlaps the other half
            # (ar2_full: single full-width AR2 — fewer collectives in flight)
            ps_a = sweep_half(False, (0, 1), "a")
            if not ar2_full:
                allreduce(ps_a[0:32], 1.0, rows=slice(0, 32))
            else:
                ar_src = ar32_sb if ar_f32 else ar16_sb
                nc.vector.tensor_scalar(
                    ar_src[0:32], ps_a[0:32], 1.0, None, mybir.AluOpType.mult
                )
            if phases <= 1:
                nc.sync.dma_start(out=out_ext[:], in_=s16_sb[:])
                return
            ps_b = sweep_half(False, (2, 3), "b")
            if not ar2_full:
                allreduce(ps_b[0:32], 1.0, rows=slice(32, 64))
            else:
                ar_src = ar32_sb if ar_f32 else ar16_sb
                nc.vector.tensor_scalar(
                    ar_src[32:64], ps_b[0:32], 1.0, None, mybir.AluOpType.mult
                )
                allreduce_pre(rows=slice(0, 64))
            squash(rows=slice(0, 32))
            bcast_v((0, 1))
            if phases <= 2:
                nc.sync.dma_start(out=out_ext[:], in_=s16_sb[:])
                return
            ps_c = sweep_half(True, (0, 1), "c")
            nc.vector.tensor_copy(s_sb[0:32], ps_c[0:32])
            nc.sync.dma_start(out=out_ext[0:32], in_=s_sb[0:32])
            squash(rows=slice(32, 64))
            bcast_v((2, 3))
            if phases <= 3:
                nc.sync.dma_start(out=out_ext[32:64], in_=s16_sb[32:64])
                return
            ps_d = sweep_half(True, (2, 3), "d")
            nc.vector.tensor_copy(s_sb[32:64], ps_d[0:32])
            nc.sync.dma_start(out=out_ext[32:64], in_=s_sb[32:64])

        if hwloop > 1:
            with tc.For_i(0, hwloop):
                one_pass()
        else:
            for _rep in range(repeat):
                one_pass()

    lowp.__exit__(None, None, None)
    _split_multi_waits(nc)
    return nc


def host_prep(inputs, W, core):
    n0 = core * NLOC
    Wk = np.ascontiguousarray(W[:, n0:n0 + NLOC])          # [C, 144, Dc, Di]
    xk = np.ascontiguousarray(inputs[:, n0:n0 + NLOC])     # [B, 144, Di]

    # w_pack[g, n8*16+i, j*32+c] = W[c, g*8+n8, j, i]  (j-major free dim)
    wg = Wk.reshape(C, NG, 8, Dc, Di)                      # c g n8 j i
    w_pack = np.ascontiguousarray(
        wg.transpose(1, 2, 4, 3, 0).reshape(NG, 128, 1024)
    ).astype(_DT["np"])

    # x arranged [g, n8, i, b]
    xg = xk.reshape(B, NG, 8, Di).transpose(1, 2, 3, 0)    # g n8 i b
    # x_dense[(n8,i), g*64+b]
    x_dense = np.ascontiguousarray(
        xg.reshape(NG, 128, 64).transpose(1, 0, 2).reshape(128, NG * 64)
    ).astype(_DT["np"])

    # x_bd[g, (n8,i), q*128 + (n8,b16)] block-diagonal
    x_bd = np.zeros((NG, 128, NQ * 128), dtype=_DT["np"])
    for n8 in range(8):
        for q in range(NQ):
            x_bd[:, n8 * 16:(n8 + 1) * 16,
                 q * 128 + n8 * 16:q * 128 + (n8 + 1) * 16] = \
                xg[:, n8, :, q * 16:(q + 1) * 16]

    # delta[(n8,bq), q*64 + (q*16+bq)]
    delta = np.zeros((128, NQ * 64), dtype=_DT["np"])
    for q in range(NQ):
        for n8 in range(8):
            for bq in range(16):
                delta[n8 * 16 + bq, q * 64 + q * 16 + bq] = 1.0

    return {"w_pack": w_pack, "x_bd": x_bd, "x_dense": x_dense, "delta": delta}


def postprocess(partials):
    """Sum per-core s2 partials [64, 1024 (c-major)], squash, reshape to
    [B, C, Dc]."""
    s = np.sum(np.stack([np.asarray(p, np.float32) for p in partials]), axis=0)
    s = s.reshape(B, Dc, C).transpose(0, 2, 1)
    n2 = np.sum(s * s, axis=-1, keepdims=True)
    v = s * (n2 / (1.0 + n2) / np.sqrt(n2 + EPS))
    return np.ascontiguousarray(v.astype(np.float32))


_NC_CACHE = {}


def _get_nc():
    if "nc" not in _NC_CACHE:
        _NC_CACHE["nc"] = build_program()
    return _NC_CACHE["nc"]


def _run_device(inputs, W, _trace=False):
    nc = _get_nc()
    in_maps = [host_prep(inputs, W, k) for k in range(NCORES)]
    res = run_bass_kernel_spmd(
        nc, in_maps, core_ids=list(range(NCORES)), trace=_trace
    )
    kernel.last_results = res
    return postprocess([res.results[k]["out"] for k in range(NCORES)])


def _run_subprocess(inputs, W):
    """Retry path: the collective path very occasionally hits a runtime race
    (NRT_EXEC_UNIT_UNRECOVERABLE) that poisons the in-process mesh; a fresh
    process recovers.  Ship inputs/outputs through a temp npz."""
    import subprocess, sys, tempfile, os
    d = tempfile.mkdtemp()
    np.savez(os.path.join(d, "in.npz"), inputs=inputs, W=W)
    code = (
        "import numpy as np, importlib.util, sys\n"
        f"spec = importlib.util.spec_from_file_location('kmod', {__file__!r})\n"
        "m = importlib.util.module_from_spec(spec); spec.loader.exec_module(m)\n"
        f"d = np.load({os.path.join(d, 'in.npz')!r})\n"
        "out = m._run_device(d['inputs'], d['W'])\n"
        f"np.save({os.path.join(d, 'out.npy')!r}, out)\n"
    )
    subprocess.run([sys.executable, "-c", code], check=True, timeout=1800)
    return np.load(os.path.join(d, "out.npy"))


def kernel(inputs, W, _trace=False):
    inputs = np.asarray(inputs, dtype=np.float32)
    W = np.asarray(W, dtype=np.float32)
    try:
        return _run_device(inputs, W, _trace=_trace)
    except Exception:
        pass
    last = None
    for _ in range(3):
        try:
            return _run_subprocess(inputs, W)
        except Exception as e:
            last = e
    raise last


if __name__ == "__main__":
    rng = np.random.default_rng(0)
    x = rng.normal(size=(B, N, Di)).astype(np.float32)
    w = (rng.normal(size=(C, N, Dc, Di)) / np.sqrt(Di)).astype(np.float32)
    out = kernel(x, w)
    print("out", out.shape, out.dtype, np.abs(out).max())
